# revision 1
# baseline (speedup 1.0000x reference)
"""DIoU loss (mean) on 8 Trainium2 NeuronCores via Bass/Tile.

Sharding: boxes [2e6, 4] are viewed as [128, 15625, 4] (partition-major)
and the 15625 columns are split across 8 cores (1956 cols/core, the tail
padded with identity boxes whose contribution is subtracted on the host).
Each core computes per-partition partial sums of iou and cd/diag; the
host finishes the mean in float64.

Per-box math (per axis a, with p1/p2/t1/t2 the box edges):
  d1 = p1-t1, d2 = p2-t2                      (delta quad z)
  h  = |d1|+|d2|,  g = (p2-p1)+(t2-t1)
  2u = g-h (overlap*2),  2e = g+h (enclosing extent*2),  d = d1+d2 (2*center diff)
  inter4 = relu(2u_x)*relu(2u_y) = 4*inter;  union4 = 4*(area_p+area_t) - inter4
  diag4 = (2e_x)^2+(2e_y)^2 = 4*diag;  cd4 = d_x^2+d_y^2 = 4*cd
  loss_i = 1 - inter/union + cd/diag = 1 - inter4/union4... (4x cancels)
Work is split DVE/ACT/GPSIMD to balance engine busy time while keeping
the iou critical chain on DVE/ACT (GPSIMD only feeds side branches).
"""

import numpy as np

import concourse.bass as bass
import concourse.mybir as mybir
from concourse import bacc
from concourse.tile import TileContext
from concourse.bass_utils import run_bass_kernel_spmd

N_BOXES = 2_000_000
P = 128
COLS = N_BOXES // P            # 15625
N_CORES = 8
W = 1956                       # columns per core (8*1956 = 15648 >= 15625)
NCH = 4                        # chunks per core (when no explicit list)
CHUNKS = [120, 306, 306, 306, 306, 306, 306]  # small head chunk cuts pipeline fill
PAD_BOXES = N_CORES * W * P - N_BOXES  # 2944

F32 = mybir.dt.float32
ALU = mybir.AluOpType
AF = mybir.ActivationFunctionType

_CACHE = {}


def _register_custom_ops():
    """Register fused DVE ops (idempotent); self-pin uops_sha."""
    import concourse.dve_ops as dve_ops_mod
    from concourse.dve_spec import Spec, Src0, Src1, Zero, maxx, relu, sq, lower
    from concourse.dve_ops import OPS, DveOp, has_src1
    from concourse.dve_uop import DveOpSpec

    def reg(name, spec):
        for op in OPS:
            if op.name == name:
                return op
        op = DveOp(name, spec, subdim=False, uops_sha={})
        OPS.append(op)
        row = dve_ops_mod._CUSTOM_DVE_ROW_BASE + len(OPS) - 1
        assert row < 0x20, "custom-DVE row field overflow"
        dve_ops_mod._SUB_OPCODE_FOR_NAME[name] = row
        dve_ops_mod.CUSTOM_DVE_SPECS[name] = spec
        for ver in ("v3", "v4"):
            sp = DveOpSpec(name=name, opcode=row, uops=lower(spec, ver=ver),
                           rd1_en=has_src1(spec))
            op.uops_sha[ver] = sp.sha(ver)
        return op

    abs2sum = reg("ANT_ABS2SUM", Spec(
        body=maxx(Src0, Zero - Src0) + maxx(Src1, Zero - Src1),
        reference=lambda in0, in1: np.abs(in0) + np.abs(in1)))
    relumul = reg("ANT_RELUMUL", Spec(
        body=relu(Src0) * relu(Src1),
        reference=lambda in0, in1: np.maximum(in0, 0) * np.maximum(in1, 0)))
    sq2sum = reg("ANT_SQ2SUM", Spec(
        body=sq(Src0) + sq(Src1),
        reference=lambda in0, in1: in0 * in0 + in1 * in1))
    return abs2sum, relumul, sq2sum



def _build_program(nch=NCH, bio=2, bwk=2, bsg=2, chunks=None, act_recip=False, act_rd=False, diag_dve=False, lag=1, area_dve=True, dma_e2=False, swdge_head=False, tail_gp=False):
    # ramped chunk sizes: small chunks at both ends shorten pipeline
    # fill/drain; interior chunks are large to amortize per-op overhead
    if chunks is None:
        fc = W // nch
        chunks = [fc] * nch
        chunks[-1] = W - fc * (nch - 1)
    nch = len(chunks)
    offs = [sum(chunks[:i]) for i in range(nch)]
    fcmax = max(chunks)
    nc = bacc.Bacc(None, target_bir_lowering=False)

    pred_d = nc.dram_tensor("pred", [P, W, 4], F32, kind="ExternalInput")
    targ_d = nc.dram_tensor("targ", [P, W, 4], F32, kind="ExternalInput")
    acc_d = nc.dram_tensor("acc", [P, nch], F32, kind="ExternalOutput")

    dve = nc.vector
    gp = nc.gpsimd
    ABS2SUM, RELUMUL, SQ2SUM = _register_custom_ops()

    with TileContext(nc) as tc:
        with (
            tc.tile_pool(name="io", bufs=bio) as io,
            tc.tile_pool(name="wk", bufs=bwk) as wk,
            tc.tile_pool(name="sg", bufs=bsg) as sg,
            tc.tile_pool(name="accp", bufs=1) as accp,
        ):
            acc = accp.tile([P, nch], F32)
            state = {}

            def front(i):
                fc = chunks[i]
                o0 = offs[i]
                pt = io.tile([P, fc, 4], F32, tag="pred")
                tt = io.tile([P, fc, 4], F32, tag="targ")
                eng = gp if (i == 0 and swdge_head) else nc.sync
                eng.dma_start(out=pt[:], in_=pred_d[:, o0:o0 + fc, :])
                eng.dma_start(out=tt[:], in_=targ_d[:, o0:o0 + fc, :])

                # box extents first: each needs only one of the two DMAs,
                # so GPSIMD can start before both loads complete
                ap = wk.tile([P, fc, 2], F32, tag="ap")
                gp.tensor_sub(ap[:], pt[:, :, 2:4], pt[:, :, 0:2])
                at = wk.tile([P, fc, 2], F32, tag="at")
                gp.tensor_sub(at[:], tt[:, :, 2:4], tt[:, :, 0:2])

                # delta quad: (d1x, d1y, d2x, d2y) = pred - targ
                z = wk.tile([P, fc, 4], F32, tag="z")
                dve.tensor_sub(z[:], pt[:], tt[:])

                # d = d1 + d2 = 2*(center diff), per axis
                dct = wk.tile([P, fc, 2], F32, tag="dct")
                gp.tensor_add(dct[:], z[:, :, 0:2], z[:, :, 2:4])

                g = wk.tile([P, fc, 2], F32, tag="g")
                dve.tensor_add(g[:], ap[:], at[:])
                # h = |d1| + |d2| fused on DVE
                h = wk.tile([P, fc, 2], F32, tag="h")
                dve._custom_dve(ABS2SUM, out=h[:], in0=z[:, :, 0:2], in1=z[:, :, 2:4])

                # 2*u (unclipped overlap) and 2*e (enclosing extent);
                # e2 = g + h built on the idle DMA engines (copy + accum)
                u2 = wk.tile([P, fc, 2], F32, tag="u2")
                dve.tensor_sub(u2[:], g[:], h[:])
                e2t = wk.tile([P, fc, 2], F32, tag="e2t")
                if dma_e2:
                    nc.sync.dma_start(out=e2t[:], in_=g[:])
                    gp.dma_start(out=e2t[:], in_=h[:], accum_op=ALU.add)
                else:
                    gp.tensor_add(e2t[:], g[:], h[:])

                sqe = wk.tile([P, fc, 2], F32, tag="sqe")
                nc.scalar.activation(sqe[:], e2t[:], AF.Square)
                sqd = wk.tile([P, fc, 2], F32, tag="sqd")
                nc.scalar.activation(sqd[:], dct[:], AF.Square)
                state[i] = (ap, at, u2, sqe, sqd)

            def recip(dst, src, scratch_tag):
                if act_recip:
                    t = sg.tile(list(src.shape), F32, tag=scratch_tag)
                    nc.scalar.activation(t[:], src[:], AF.Ln)
                    nc.scalar.activation(dst[:], t[:], AF.Exp, scale=-1.0)
                else:
                    dve.reciprocal_approx_fast(out=dst[:], in_=src[:])

            def back(i):
                fc = chunks[i]
                ap, at, u2, sqe, sqd = state.pop(i)
                # numerator pair IC = (inter4, -cd4); denominator pair UD =
                # (union4, diag4). One reciprocal and one accumulating stt
                # then yield sum(iou - cd/diag) directly.
                ic = sg.tile([P, fc, 2], F32, tag="ic")
                dve._custom_dve(RELUMUL, out=ic[:, :, 0],
                                in0=u2[:, :, 0], in1=u2[:, :, 1])
                last = tail_gp and i >= nch - tail_gp
                areap = sg.tile([P, fc], F32, tag="areap")
                (gp if (last or not area_dve) else dve).tensor_mul(
                    areap[:], ap[:, :, 0], ap[:, :, 1])
                areat = sg.tile([P, fc], F32, tag="areat")
                gp.tensor_mul(areat[:], at[:, :, 0], at[:, :, 1])
                asum = sg.tile([P, fc], F32, tag="asum")
                (gp if last else dve).tensor_add(asum[:], areap[:], areat[:])
                ud = sg.tile([P, fc, 2], F32, tag="ud")
                dve.scalar_tensor_tensor(
                    out=ud[:, :, 0], in0=asum[:], scalar=4.0, in1=ic[:, :, 0],
                    op0=ALU.mult, op1=ALU.subtract)
                # -cd4 = -dx^2 - dy^2
                dve.scalar_tensor_tensor(
                    out=ic[:, :, 1], in0=sqd[:, :, 0], scalar=-1.0,
                    in1=sqd[:, :, 1], op0=ALU.mult, op1=ALU.subtract)
                (dve if diag_dve else gp).tensor_add(
                    ud[:, :, 1], sqe[:, :, 0], sqe[:, :, 1])
                rud = sg.tile([P, fc, 2], F32, tag="rud")
                dve.reciprocal_approx_fast(out=rud[:], in_=ud[:])

                scr = sg.tile([P, fc, 2], F32, tag="scr")
                dve.scalar_tensor_tensor(
                    out=scr[:], in0=ic[:], scalar=1.0, in1=rud[:],
                    op0=ALU.mult, op1=ALU.mult, accum_out=acc[:, i:i + 1],
                )

            for i in range(nch + lag):
                if i < nch:
                    front(i)
                if i >= lag:
                    back(i - lag)

            nc.sync.dma_start(out=acc_d[:], in_=acc[:])

    nc.finalize()
    return nc


def _shard(arr):
    """arr [N_BOXES, 4] -> list of 8 per-core [P, W, 4] arrays (tail padded)."""
    v = np.ascontiguousarray(arr, dtype=np.float32).reshape(P, COLS, 4)
    pad_cols = N_CORES * W - COLS
    dummy = np.tile(
        np.array([0.0, 0.0, 1.0, 1.0], dtype=np.float32), (P, pad_cols, 1)
    )
    full = np.concatenate([v, dummy], axis=1)
    return [np.ascontiguousarray(full[:, c * W:(c + 1) * W, :]) for c in range(N_CORES)]


def kernel(pred_boxes, target_boxes):
    if "nc" not in _CACHE:
        _CACHE["nc"] = _build_program(chunks=CHUNKS, bwk=3, tail_gp=1)
        _CACHE["nch"] = len(CHUNKS)
    nc = _CACHE["nc"]

    preds = _shard(np.asarray(pred_boxes))
    targs = _shard(np.asarray(target_boxes))
    in_maps = [{"pred": preds[c], "targ": targs[c]} for c in range(N_CORES)]

    # the device occasionally reports a transient NRT_EXEC_UNIT_UNRECOVERABLE
    # wedge; it clears on re-execution, so retry a few times
    last_err = None
    for _attempt in range(4):
        try:
            res = run_bass_kernel_spmd(nc, in_maps, list(range(N_CORES)))
            break
        except Exception as e:
            last_err = e
    else:
        raise last_err

    # each acc column already holds sum(iou - cd/diag) for one chunk
    s = 0.0
    for c in range(N_CORES):
        s += res.results[c]["acc"].astype(np.float64).sum()
    # padded identity boxes contribute iou-ratio = 1 each
    s -= float(PAD_BOXES)
    loss = 1.0 - s / float(N_BOXES)
    return np.float32(loss)

